# revision 5
# baseline (speedup 1.0000x reference)
"""GRU Trainium kernel v4: fp8 DoubleRow r/z + bf16 n-gate, HW-valid banks.

Per-core problem: B=32, T steps, H=512, 2 layers, gates [r,z,n].

v4 over v3 (HW constraint: DoubleRow matmuls may only write psum
partitions 0:32):
  - Each of r0/r1/z0/z1 gets its OWN psum bank with the DR output at
    partitions 0:32. Four per-gate sigmoids write into the shared rz tile
    at the v3 region layout (r0@0 r1@32 zbar0@64 zbar1@96) so the rest of
    the chain is unchanged.
  - ni accumulations fold into the nh half-banks (B0 cols 0:256,
    B1 cols 256:512; regions ni0@0 ni1@32 nh0@64 nh1@96), one K=3
    one-hot bias init per half-bank.
  - n-path H-split, two alternating transpose banks, fp8 ring derived
    from the bf16 ring on Pool: as v3.
  Scaling: fp8 W and h x8 -> psum 64x; bf16 weights/biases 64x;
  activations divide by 64 via imm scale.
"""
import numpy as np
import ml_dtypes
import concourse.bass as bass
from concourse import bacc
import concourse.tile as tile
import concourse.mybir as mybir

F32 = mybir.dt.float32
BF16 = mybir.dt.bfloat16
FP8 = mybir.dt.float8e4
AF = mybir.ActivationFunctionType
ALU = mybir.AluOpType
DR = mybir.MatmulPerfMode.DoubleRow

H = 512
HH = 256
BL = 32
NK = 4
NP = 2
RING = 4
SCALE = 8.0
ISCALE = 1.0 / (SCALE * SCALE)


def build_gru(T=512, skew=2, n_cores=8):
    nc = bacc.Bacc("TRN2", target_bir_lowering=False, debug=False,
                   num_devices=n_cores)
    xw_d = nc.dram_tensor("xw", (2, T * BL), BF16, kind="ExternalInput").ap()
    w8h0_d = nc.dram_tensor("w8h0", (128, 8 * H), FP8, kind="ExternalInput").ap()
    w8h1_d = nc.dram_tensor("w8h1", (128, 8 * H), FP8, kind="ExternalInput").ap()
    w8i1_d = nc.dram_tensor("w8i1", (128, 8 * H), FP8, kind="ExternalInput").ap()
    wnb_d = nc.dram_tensor("wnb", (128, 12 * H), BF16, kind="ExternalInput").ap()
    wx2_d = nc.dram_tensor("wx2", (2, 3 * H), BF16, kind="ExternalInput").ap()
    biasA_d = nc.dram_tensor("biasA", (1, 2 * H), BF16, kind="ExternalInput").ap()
    biasB3_d = nc.dram_tensor("biasB3", (3, H), BF16, kind="ExternalInput").ap()
    wfc_d = nc.dram_tensor("wfc", (128, 8), BF16, kind="ExternalInput").ap()
    fcb_d = nc.dram_tensor("fcb", (1, 2), BF16, kind="ExternalInput").ap()
    onehotB3_d = nc.dram_tensor("onehotB3", (3, 128), BF16, kind="ExternalInput").ap()
    out_d = nc.dram_tensor("out", (BL, 2), F32, kind="ExternalOutput").ap()

    with tile.TileContext(nc) as tc:
        import contextlib
        with contextlib.ExitStack() as ctx:
            const = ctx.enter_context(tc.tile_pool(name="const", bufs=1))
            state = ctx.enter_context(tc.tile_pool(name="state", bufs=1))
            scratch = ctx.enter_context(tc.tile_pool(name="scratch", bufs=2))
            psp = ctx.enter_context(tc.tile_pool(name="psp", bufs=1, space="PSUM"))

            # ---- persistent tiles ----
            xw = const.tile([2, T * BL], BF16)
            w8h0 = const.tile([128, 8 * H], FP8)
            w8h1 = const.tile([128, 8 * H], FP8)
            w8i1 = const.tile([128, 8 * H], FP8)
            wnb = const.tile([128, 12 * H], BF16)
            wx2 = const.tile([2, 3 * H], BF16)
            biasA = const.tile([1, 2 * H], BF16)
            biasB3 = const.tile([3, H], BF16)
            wfc = const.tile([128, 8], BF16)
            fcb = const.tile([1, 2], BF16)
            onehotB3 = const.tile([3, 128], BF16)  # rows: nh0@64, nh1@96, ni1@32
            for t_, d_ in [(xw, xw_d), (w8h0, w8h0_d), (w8h1, w8h1_d),
                           (w8i1, w8i1_d), (wnb, wnb_d), (wx2, wx2_d),
                           (biasA, biasA_d), (biasB3, biasB3_d),
                           (wfc, wfc_d), (fcb, fcb_d),
                           (onehotB3, onehotB3_d)]:
                nc.sync.dma_start(out=t_[:], in_=d_)

            id4 = const.tile([128, 32], BF16)
            from concourse.masks import make_identity
            for j_ in range(4):
                make_identity(nc, id4[32 * j_:32 * (j_ + 1), :])

            ones_t = const.tile([1, BL], BF16)
            nc.vector.memset(ones_t[:], 1.0)
            ones_lhs = ones_t[0:1, 0:BL]

            zeros_rhs_t = const.tile([1, H], BF16)
            nc.vector.memset(zeros_rhs_t[:], 0.0)
            zeros_rhs = zeros_rhs_t[0:1, :]

            # ---- state ----
            h_sb = [state.tile([64, H], BF16, name=f"h{i}", tag=f"h{i}")
                    for i in range(2)]
            hT8 = [[[state.tile([128, NP * BL], FP8, name=f"hT8_{l}_{p}_{r}",
                                tag=f"hT8_{l}_{p}_{r}") for r in range(RING)]
                    for p in range(NP)] for l in range(2)]
            hTb = [[state.tile([128, NK * BL], BF16, name=f"hTb_{l}_{r}",
                               tag=f"hTb_{l}_{r}") for r in range(RING)]
                   for l in range(2)]
            for t_ in h_sb:
                nc.vector.memset(t_[:], 0.0)
            for l in range(2):
                for p in range(NP):
                    for r in range(RING):
                        nc.vector.memset(hT8[l][p][r][:], 0.0)
                for r in range(RING):
                    nc.vector.memset(hTb[l][r][:], 0.0)

            def w8sl(w, g, p):
                base = (g * NP + p) * 2 * H
                return w[:, base:base + 2 * H].rearrange(
                    "p (two n) -> p two n", two=2)

            def wnsl(m, c, half):
                base = (m * NK + c) * H
                return wnb[:, base + half * HH:base + half * HH + HH]

            def hT8ap(l, p, r):
                return hT8[l][p][r][:, :].rearrange(
                    "p (two m) -> p two m", two=2)

            # ---------------- superstep loop ----------------
            n_super = T + skew
            banks = {}

            def emit_early(s):
                """x-side / bias / wi1 matmuls (no dep on chain s-1)."""
                if s >= n_super:
                    return
                l0 = s < T
                l1 = s >= skew
                t0, t1 = s, s - skew
                # per-gate DR banks (DR output must be partitions 0:32)
                b_r0 = psp.tile([128, H], F32, tag="b_r0")
                b_r1 = psp.tile([128, H], F32, tag="b_r1")
                b_z0 = psp.tile([128, H], F32, tag="b_z0")
                b_z1 = psp.tile([128, H], F32, tag="b_z1")
                bB0 = psp.tile([128, H], F32, tag="bB0")
                bB1 = psp.tile([128, H], F32, tag="bB1")
                banks[s] = (b_r0, b_r1, b_z0, b_z1, bB0, bB1)

                if l0:
                    xt = xw[0:2, BL * t0: BL * (t0 + 1)]
                    nc.tensor.matmul(b_r0[0:32, :], lhsT=xt,
                                     rhs=wx2[0:2, 0:H], start=True, stop=False,
                                     tile_position=(0, 0), skip_group_check=True)
                    nc.tensor.matmul(b_z0[0:32, :], lhsT=xt,
                                     rhs=wx2[0:2, H:2 * H], start=True,
                                     stop=False, tile_position=(0, 0),
                                     skip_group_check=True)
                else:
                    nc.tensor.matmul(b_r0[0:32, :], lhsT=ones_lhs,
                                     rhs=zeros_rhs, start=True, stop=True,
                                     tile_position=(0, 0), skip_group_check=True)
                    nc.tensor.matmul(b_z0[0:32, :], lhsT=ones_lhs,
                                     rhs=zeros_rhs, start=True, stop=True,
                                     tile_position=(0, 0), skip_group_check=True)
                # l1 r/z banks: bias init + wi1 DR (early)
                nc.tensor.matmul(b_r1[0:32, :], lhsT=ones_lhs,
                                 rhs=biasA[0:1, 0:H], start=True,
                                 stop=(not l1), tile_position=(0, 0),
                                 skip_group_check=True)
                nc.tensor.matmul(b_z1[0:32, :], lhsT=ones_lhs,
                                 rhs=biasA[0:1, H:2 * H], start=True,
                                 stop=(not l1), tile_position=(0, 0),
                                 skip_group_check=True)
                if l1:
                    for p in range(NP):
                        nc.tensor.matmul(b_r1[0:32, :],
                                         lhsT=hT8ap(0, p, t1 % RING),
                                         rhs=w8sl(w8i1, 0, p), start=False,
                                         stop=False, perf_mode=DR,
                                         tile_position=(0, 0),
                                         skip_group_check=True)
                    for p in range(NP):
                        nc.tensor.matmul(b_z1[0:32, :],
                                         lhsT=hT8ap(0, p, t1 % RING),
                                         rhs=w8sl(w8i1, 1, p), start=False,
                                         stop=False, perf_mode=DR,
                                         tile_position=(0, 0),
                                         skip_group_check=True)
                # B half-banks: one-hot bias init (ni1/nh0/nh1) + x-n + wi1n
                for half, bBh in [(0, bB0), (1, bB1)]:
                    hs = slice(half * HH, half * HH + HH)
                    nc.tensor.matmul(bBh[:, 0:HH], lhsT=onehotB3[:, :],
                                     rhs=biasB3[:, hs], start=True, stop=False,
                                     tile_position=(0, 0),
                                     skip_group_check=True)
                    if l0:
                        xt = xw[0:2, BL * t0: BL * (t0 + 1)]
                        nc.tensor.matmul(bBh[0:32, 0:HH], lhsT=xt,
                                         rhs=wx2[0:2, 2 * H + half * HH:
                                                 2 * H + half * HH + HH],
                                         start=False, stop=False,
                                         tile_position=(0, 0),
                                         skip_group_check=True)
                    if l1:
                        hb = hTb[0][t1 % RING]
                        for c in range(NK):
                            nc.tensor.matmul(bBh[32:64, 0:HH],
                                             lhsT=hb[:, BL * c:BL * (c + 1)],
                                             rhs=wnsl(2, c, half),
                                             start=False, stop=False,
                                             tile_position=(0, 32),
                                             skip_group_check=True)

            def emit_late(s):
                """wh0/wh1 matmuls (depend on superstep s-1's ring copies);
                set the stop flags."""
                l0 = s < T
                l1 = s >= skew
                t0, t1 = s, s - skew
                b_r0, b_r1, b_z0, b_z1, bB0, bB1 = banks[s]
                # r/z DR mms, pair-major
                groups = []  # (bank, lhsT-list)
                if l0 and t0 >= 1:
                    groups.append((b_r0, [(hT8ap(0, p, (t0 - 1) % RING),
                                           w8sl(w8h0, 0, p)) for p in range(NP)]))
                    groups.append((b_z0, [(hT8ap(0, p, (t0 - 1) % RING),
                                           w8sl(w8h0, 1, p)) for p in range(NP)]))
                elif l0:
                    groups.append((b_r0, []))
                    groups.append((b_z0, []))
                if l1 and t1 >= 1:
                    groups.append((b_r1, [(hT8ap(1, p, (t1 - 1) % RING),
                                           w8sl(w8h1, 0, p)) for p in range(NP)]))
                    groups.append((b_z1, [(hT8ap(1, p, (t1 - 1) % RING),
                                           w8sl(w8h1, 1, p)) for p in range(NP)]))
                elif l1:
                    groups.append((b_r1, []))
                    groups.append((b_z1, []))
                for bank, mms in groups:
                    if not mms:
                        nc.tensor.matmul(bank[0:32, :], lhsT=ones_lhs,
                                         rhs=zeros_rhs, start=False, stop=True,
                                         tile_position=(0, 0),
                                         skip_group_check=True)
                        continue
                    for i, (lh, rh) in enumerate(mms):
                        nc.tensor.matmul(bank[0:32, :], lhsT=lh, rhs=rh,
                                         start=False, stop=(i == len(mms) - 1),
                                         perf_mode=DR, tile_position=(0, 0),
                                         skip_group_check=True)
                # nh halves (bf16)
                for half, bBh in [(0, bB0), (1, bB1)]:
                    b_mms = []
                    if l0 and t0 >= 1:
                        hb = hTb[0][(t0 - 1) % RING]
                        for c in range(NK):
                            b_mms.append((hb[:, BL * c:BL * (c + 1)],
                                          wnsl(0, c, half), 64))
                    if l1 and t1 >= 1:
                        hb = hTb[1][(t1 - 1) % RING]
                        for c in range(NK):
                            b_mms.append((hb[:, BL * c:BL * (c + 1)],
                                          wnsl(1, c, half), 96))
                    for i, (lh, rh, ps) in enumerate(b_mms):
                        nc.tensor.matmul(bBh[ps:ps + 32, 0:HH],
                                         lhsT=lh, rhs=rh, start=False,
                                         stop=(i == len(b_mms) - 1),
                                         tile_position=(0, ps),
                                         skip_group_check=True)
                    if not b_mms:
                        nc.tensor.matmul(bBh[0:32, 0:HH], lhsT=ones_lhs,
                                         rhs=zeros_rhs[0:1, 0:HH],
                                         start=False, stop=True,
                                         tile_position=(0, 0),
                                         skip_group_check=True)

            emit_early(0)
            for s in range(n_super):
                l0 = s < T
                l1 = s >= skew
                t0, t1 = s, s - skew
                par = s % 2
                b_r0, b_r1, b_z0, b_z1, bB0, bB1 = banks[s]

                emit_late(s)

                rz = scratch.tile([128, H], BF16, tag="rz")
                tmp = scratch.tile([64, H], BF16, tag="tmp")
                u = scratch.tile([64, H], BF16, tag="u")
                nn_ = scratch.tile([64, H], BF16, tag="nn")
                dd = scratch.tile([128, H], BF16, tag="dd")
                ee = scratch.tile([64, H], BF16, tag="ee")
                hnew = h_sb[par]
                hold = h_sb[1 - par]

                # per-gate sigmoids into the shared rz layout
                # (r0@0 r1@32 zbar0@64 zbar1@96)
                if l0:
                    nc.scalar.activation(rz[0:32, :], b_r0[0:32, :],
                                         AF.Sigmoid, scale=ISCALE)
                    nc.scalar.activation(rz[64:96, :], b_z0[0:32, :],
                                         AF.Sigmoid, scale=-ISCALE)
                if l1:
                    nc.scalar.activation(rz[32:64, :], b_r1[0:32, :],
                                         AF.Sigmoid, scale=ISCALE)
                    nc.scalar.activation(rz[96:128, :], b_z1[0:32, :],
                                         AF.Sigmoid, scale=-ISCALE)
                lo, hi = (0 if l0 else 32), (64 if l1 else 32)

                halves = [(0, bB0), (1, bB1)]
                # DVE: tmp/u straight from psum, 1/64 scale folded in
                for half, bBh in halves:
                    hc = slice(half * HH, half * HH + HH)
                    nc.vector.scalar_tensor_tensor(
                        out=tmp[lo:hi, hc], in0=bBh[64 + lo:64 + hi, 0:HH],
                        scalar=ISCALE, in1=rz[lo:hi, hc],
                        op0=ALU.mult, op1=ALU.mult)
                    nc.vector.scalar_tensor_tensor(
                        out=u[lo:hi, hc], in0=bBh[lo:hi, 0:HH],
                        scalar=ISCALE, in1=tmp[lo:hi, hc],
                        op0=ALU.mult, op1=ALU.add)
                # ACT: tanh per half; Pool: d, e; DVE: h'
                for half, _ in halves:
                    hc = slice(half * HH, half * HH + HH)
                    nc.scalar.activation(nn_[lo:hi, hc], u[lo:hi, hc], AF.Tanh)
                    nc.gpsimd.tensor_sub(out=dd[64 + lo:64 + hi, hc],
                                         in0=nn_[lo:hi, hc],
                                         in1=hold[lo:hi, hc])
                    nc.gpsimd.tensor_mul(out=ee[lo:hi, hc],
                                         in0=dd[64 + lo:64 + hi, hc],
                                         in1=rz[64 + lo:64 + hi, hc])
                    nc.vector.tensor_add(out=hnew[lo:hi, hc],
                                         in0=ee[lo:hi, hc],
                                         in1=hold[lo:hi, hc])

                # ---- early matmuls of next superstep (keep PE busy) ----
                emit_early(s + 1)

                # ---- transposes + ring copies, per half ----
                trpsX = psp.tile([128, 1024], BF16, tag="trpsX")
                trpsY = psp.tile([128, 1024], BF16, tag="trpsY")
                for half, _ in halves:
                    for (active, base, l) in [(l0, 0, 0), (l1, 32, 1)]:
                        if not active:
                            continue
                        tstep = t0 if l == 0 else t1
                        blk = (l + half) % 2          # X, Y, Y, X
                        tr = trpsX if blk == 0 else trpsY
                        boff = half * 256
                        for j, c in enumerate((2 * half, 2 * half + 1)):
                            nc.tensor.transpose(
                                tr[:, boff + BL * j: boff + BL * (j + 1)],
                                hnew[base:base + 32, 128 * c:128 * (c + 1)],
                                id4[base:base + 32, :],
                                tile_position=(base, 0),
                            )
                        src = tr[:, boff:boff + 2 * BL]
                        dstb = hTb[l][tstep % RING][:, 2 * BL * half:
                                                    2 * BL * (half + 1)]
                        if l == 0:
                            nc.scalar.activation(dstb, src, AF.Copy)
                        else:
                            nc.vector.tensor_copy(out=dstb, in_=src)
                        # fp8 ring from the bf16 ring (SBUF), on Pool
                        nc.gpsimd.tensor_scalar_mul(
                            hT8[l][half][tstep % RING][:, :], dstb, SCALE)

            # ---- FC ----
            psfc = psp.tile([128, H], F32, tag="bB0")
            hfcT = hTb[1][(T - 1) % RING]
            for c in range(NK):
                nc.tensor.matmul(psfc[0:BL, 0:2],
                                 lhsT=hfcT[:, BL * c:BL * (c + 1)],
                                 rhs=wfc[:, 2 * c:2 * (c + 1)],
                                 start=(c == 0), stop=False,
                                 skip_group_check=True)
            nc.tensor.matmul(psfc[0:BL, 0:2], lhsT=ones_lhs, rhs=fcb[0:1, :],
                             start=False, stop=True, skip_group_check=True)
            out_sb = const.tile([BL, 2], F32)
            nc.vector.tensor_copy(out=out_sb[:], in_=psfc[0:BL, 0:2])
            nc.sync.dma_start(out=out_d, in_=out_sb[:])

    nc.compile()
    return nc


# ---------------- host-side packing ----------------

def pack_inputs(x, Wi0, bi0, Wi_rest, bi_rest, Wh, bh, fc_w, fc_b, n_cores=8):
    B, T = x.shape
    bl = B // n_cores
    assert bl == BL
    S = SCALE
    S2 = S * S

    def w8_pack(W3):  # r/z gates -> [128, 8*H] fp8
        a = np.empty((128, 2, NP, 2, H), np.float32)
        for g in range(2):
            for p in range(NP):
                for ko in range(2):
                    k0 = p * 256 + ko * 128
                    a[:, g, p, ko, :] = W3[g, :, k0:k0 + 128].T * S
        return a.reshape(128, 8 * H).astype(ml_dtypes.float8_e4m3)

    w8h0 = w8_pack(Wh[0])
    w8h1 = w8_pack(Wh[1])
    w8i1 = w8_pack(Wi_rest[0])

    wnb = np.empty((128, 3, NK, H), np.float32)
    for m, W3 in enumerate([Wh[0], Wh[1], Wi_rest[0]]):
        for c in range(NK):
            wnb[:, m, c, :] = W3[2, :, c * 128:(c + 1) * 128].T * S2
    wnb = wnb.reshape(128, 12 * H).astype(ml_dtypes.bfloat16)

    bias_l0 = [bi0[0] + bh[0, 0], bi0[1] + bh[0, 1], bi0[2]]
    bias_l1 = [bi_rest[0, 0] + bh[1, 0], bi_rest[0, 1] + bh[1, 1],
               bi_rest[0, 2]]
    wx2 = np.zeros((2, 3 * H), np.float32)
    for g in range(3):
        wx2[0, g * H:(g + 1) * H] = Wi0[g, :, 0] * S2
        wx2[1, g * H:(g + 1) * H] = bias_l0[g] * S2
    wx2 = wx2.astype(ml_dtypes.bfloat16)

    biasA = np.concatenate([bias_l1[0] * S2, bias_l1[1] * S2]).reshape(
        1, 2 * H).astype(ml_dtypes.bfloat16)
    # rows: nh0 (bh0n), nh1 (bh1n), ni1 (bi1n)
    biasB3 = np.stack([bh[0, 2] * S2, bh[1, 2] * S2,
                       bias_l1[2] * S2]).astype(ml_dtypes.bfloat16)

    wfc = fc_w.T.reshape(NK, 128, 2).transpose(1, 0, 2)
    wfc = np.ascontiguousarray(wfc).reshape(128, 8).astype(ml_dtypes.bfloat16)
    fcb = fc_b.reshape(1, 2).astype(ml_dtypes.bfloat16)

    onehotB3 = np.zeros((3, 128), np.float32)
    onehotB3[0, 64:96] = 1.0   # nh0
    onehotB3[1, 96:128] = 1.0  # nh1
    onehotB3[2, 32:64] = 1.0   # ni1
    onehotB3 = onehotB3.astype(ml_dtypes.bfloat16)

    in_maps = []
    for c in range(n_cores):
        xc = x[c * bl:(c + 1) * bl, :]       # [32, T]
        xw = np.empty((2, T * bl), np.float32)
        xw[0] = xc.T.reshape(-1)
        xw[1] = 1.0
        in_maps.append({
            "xw": xw.astype(ml_dtypes.bfloat16),
            "w8h0": w8h0, "w8h1": w8h1, "w8i1": w8i1, "wnb": wnb,
            "wx2": wx2, "biasA": biasA, "biasB3": biasB3,
            "wfc": wfc, "fcb": fcb, "onehotB3": onehotB3,
        })
    return in_maps


def unpack_outputs(results):
    return np.concatenate([r["out"] for r in results], axis=0)


# ---------------- public entry point ----------------
_CACHED = {}


def _get_nc(T):
    if T not in _CACHED:
        _CACHED[T] = build_gru(T=T)
    return _CACHED[T]


def kernel(x, Wi0, bi0, Wi_rest, bi_rest, Wh, bh, fc_w, fc_b):
    """Full-input 2-layer GRU (B=256, H=512) on 8 NeuronCores.

    Data-parallel over batch (32/core), weights replicated. fp8 DoubleRow
    r/z matmuls (per-gate psum banks), bf16 n-gate, H-split n-path.
    """
    from concourse.bass_utils import run_bass_kernel_spmd
    x = np.asarray(x); Wi0 = np.asarray(Wi0); bi0 = np.asarray(bi0)
    Wi_rest = np.asarray(Wi_rest); bi_rest = np.asarray(bi_rest)
    Wh = np.asarray(Wh); bh = np.asarray(bh)
    fc_w = np.asarray(fc_w); fc_b = np.asarray(fc_b)
    T = x.shape[1]
    nc = _get_nc(T)
    in_maps = pack_inputs(x, Wi0, bi0, Wi_rest, bi_rest, Wh, bh, fc_w, fc_b)
    res = run_bass_kernel_spmd(nc, in_maps, core_ids=list(range(8)))
    return unpack_outputs(res.results).astype(np.float32)


# revision 6
# speedup vs baseline: 14.9590x; 14.9590x over previous
"""GRU Trainium kernel v4: fp8 DoubleRow r/z + bf16 n-gate, HW-valid banks.

Per-core problem: B=32, T steps, H=512, 2 layers, gates [r,z,n].

v4 over v3 (HW constraint: DoubleRow matmuls may only write psum
partitions 0:32):
  - Each of r0/r1/z0/z1 gets its OWN psum bank with the DR output at
    partitions 0:32. Four per-gate sigmoids write into the shared rz tile
    at the v3 region layout (r0@0 r1@32 zbar0@64 zbar1@96) so the rest of
    the chain is unchanged.
  - ni accumulations fold into the nh half-banks (B0 cols 0:256,
    B1 cols 256:512; regions ni0@0 ni1@32 nh0@64 nh1@96), one K=3
    one-hot bias init per half-bank.
  - n-path H-split, two alternating transpose banks, fp8 ring derived
    from the bf16 ring on Pool: as v3.
  Scaling: fp8 W and h x8 -> psum 64x; bf16 weights/biases 64x;
  activations divide by 64 via imm scale.
"""
import numpy as np
import ml_dtypes
import concourse.bass as bass
from concourse import bacc
import concourse.tile as tile
import concourse.mybir as mybir

F32 = mybir.dt.float32
BF16 = mybir.dt.bfloat16
FP8 = mybir.dt.float8e4
AF = mybir.ActivationFunctionType
ALU = mybir.AluOpType
DR = mybir.MatmulPerfMode.DoubleRow

H = 512
HH = 256
BL = 32
NK = 4
NP = 2
RING = 4
SCALE = 8.0
ISCALE = 1.0 / (SCALE * SCALE)


def build_gru(T=512, skew=2, n_cores=8):
    nc = bacc.Bacc("TRN2", target_bir_lowering=False, debug=False,
                   num_devices=n_cores)
    xw_d = nc.dram_tensor("xw", (2, T * BL), BF16, kind="ExternalInput").ap()
    w8h0_d = nc.dram_tensor("w8h0", (128, 8 * H), FP8, kind="ExternalInput").ap()
    w8h1_d = nc.dram_tensor("w8h1", (128, 8 * H), FP8, kind="ExternalInput").ap()
    w8i1_d = nc.dram_tensor("w8i1", (128, 8 * H), FP8, kind="ExternalInput").ap()
    wnb_d = nc.dram_tensor("wnb", (128, 12 * H), BF16, kind="ExternalInput").ap()
    wx2_d = nc.dram_tensor("wx2", (2, 3 * H), BF16, kind="ExternalInput").ap()
    biasA_d = nc.dram_tensor("biasA", (1, 2 * H), BF16, kind="ExternalInput").ap()
    biasB3_d = nc.dram_tensor("biasB3", (3, H), BF16, kind="ExternalInput").ap()
    wfc_d = nc.dram_tensor("wfc", (128, 8), BF16, kind="ExternalInput").ap()
    fcb_d = nc.dram_tensor("fcb", (1, 2), BF16, kind="ExternalInput").ap()
    onehotB3_d = nc.dram_tensor("onehotB3", (3, 128), BF16, kind="ExternalInput").ap()
    out_d = nc.dram_tensor("out", (BL, 2), F32, kind="ExternalOutput").ap()

    with tile.TileContext(nc) as tc:
        import contextlib
        with contextlib.ExitStack() as ctx:
            const = ctx.enter_context(tc.tile_pool(name="const", bufs=1))
            state = ctx.enter_context(tc.tile_pool(name="state", bufs=1))
            scratch = ctx.enter_context(tc.tile_pool(name="scratch", bufs=2))
            psp = ctx.enter_context(tc.tile_pool(name="psp", bufs=1, space="PSUM"))

            # ---- persistent tiles ----
            xw = const.tile([2, T * BL], BF16)
            w8h0 = const.tile([128, 8 * H], FP8)
            w8h1 = const.tile([128, 8 * H], FP8)
            w8i1 = const.tile([128, 8 * H], FP8)
            wnb = const.tile([128, 12 * H], BF16)
            wx2 = const.tile([2, 3 * H], BF16)
            biasA = const.tile([1, 2 * H], BF16)
            biasB3 = const.tile([3, H], BF16)
            wfc = const.tile([128, 8], BF16)
            fcb = const.tile([1, 2], BF16)
            onehotB3 = const.tile([3, 128], BF16)  # rows: nh0@64, nh1@96, ni1@32
            for t_, d_ in [(xw, xw_d), (w8h0, w8h0_d), (w8h1, w8h1_d),
                           (w8i1, w8i1_d), (wnb, wnb_d), (wx2, wx2_d),
                           (biasA, biasA_d), (biasB3, biasB3_d),
                           (wfc, wfc_d), (fcb, fcb_d),
                           (onehotB3, onehotB3_d)]:
                nc.sync.dma_start(out=t_[:], in_=d_)

            id4 = const.tile([128, 32], BF16)
            from concourse.masks import make_identity
            for j_ in range(4):
                make_identity(nc, id4[32 * j_:32 * (j_ + 1), :])

            ones_t = const.tile([1, BL], BF16)
            nc.vector.memset(ones_t[:], 1.0)
            ones_lhs = ones_t[0:1, 0:BL]

            zeros_rhs_t = const.tile([1, H], BF16)
            nc.vector.memset(zeros_rhs_t[:], 0.0)
            zeros_rhs = zeros_rhs_t[0:1, :]

            # ---- state ----
            h_sb = [state.tile([64, H], BF16, name=f"h{i}", tag=f"h{i}")
                    for i in range(2)]
            hT8 = [[[state.tile([128, NP * BL], FP8, name=f"hT8_{l}_{p}_{r}",
                                tag=f"hT8_{l}_{p}_{r}") for r in range(RING)]
                    for p in range(NP)] for l in range(2)]
            hTb = [[state.tile([128, NK * BL], BF16, name=f"hTb_{l}_{r}",
                               tag=f"hTb_{l}_{r}") for r in range(RING)]
                   for l in range(2)]
            for t_ in h_sb:
                nc.vector.memset(t_[:], 0.0)
            for l in range(2):
                for p in range(NP):
                    for r in range(RING):
                        nc.vector.memset(hT8[l][p][r][:], 0.0)
                for r in range(RING):
                    nc.vector.memset(hTb[l][r][:], 0.0)

            def w8sl(w, g, p):
                base = (g * NP + p) * 2 * H
                return w[:, base:base + 2 * H].rearrange(
                    "p (two n) -> p two n", two=2)

            def wnsl(m, c, half):
                base = (m * NK + c) * H
                return wnb[:, base + half * HH:base + half * HH + HH]

            def hT8ap(l, p, r):
                return hT8[l][p][r][:, :].rearrange(
                    "p (two m) -> p two m", two=2)

            # ---------------- superstep loop ----------------
            n_super = T + skew
            banks = {}

            def emit_early(s):
                """x-side / bias / wi1 matmuls (no dep on chain s-1)."""
                if s >= n_super:
                    return
                l0 = s < T
                l1 = s >= skew
                t0, t1 = s, s - skew
                # per-gate DR banks (DR output must be partitions 0:32)
                b_r0 = psp.tile([128, H], F32, tag="b_r0")
                b_r1 = psp.tile([128, H], F32, tag="b_r1")
                b_z0 = psp.tile([128, H], F32, tag="b_z0")
                b_z1 = psp.tile([128, H], F32, tag="b_z1")
                bB0 = psp.tile([128, H], F32, tag="bB0")
                bB1 = psp.tile([128, H], F32, tag="bB1")
                banks[s] = (b_r0, b_r1, b_z0, b_z1, bB0, bB1)

                if l0:
                    xt = xw[0:2, BL * t0: BL * (t0 + 1)]
                    nc.tensor.matmul(b_r0[0:32, :], lhsT=xt,
                                     rhs=wx2[0:2, 0:H], start=True, stop=False,
                                     tile_position=(0, 0), skip_group_check=True)
                    nc.tensor.matmul(b_z0[0:32, :], lhsT=xt,
                                     rhs=wx2[0:2, H:2 * H], start=True,
                                     stop=False, tile_position=(0, 0),
                                     skip_group_check=True)
                else:
                    nc.tensor.matmul(b_r0[0:32, :], lhsT=ones_lhs,
                                     rhs=zeros_rhs, start=True, stop=True,
                                     tile_position=(0, 0), skip_group_check=True)
                    nc.tensor.matmul(b_z0[0:32, :], lhsT=ones_lhs,
                                     rhs=zeros_rhs, start=True, stop=True,
                                     tile_position=(0, 0), skip_group_check=True)
                # l1 r/z banks: bias init + wi1 DR (early)
                nc.tensor.matmul(b_r1[0:32, :], lhsT=ones_lhs,
                                 rhs=biasA[0:1, 0:H], start=True,
                                 stop=(not l1), tile_position=(0, 0),
                                 skip_group_check=True)
                nc.tensor.matmul(b_z1[0:32, :], lhsT=ones_lhs,
                                 rhs=biasA[0:1, H:2 * H], start=True,
                                 stop=(not l1), tile_position=(0, 0),
                                 skip_group_check=True)
                if l1:
                    for p in range(NP):
                        nc.tensor.matmul(b_r1[0:32, :],
                                         lhsT=hT8ap(0, p, t1 % RING),
                                         rhs=w8sl(w8i1, 0, p), start=False,
                                         stop=False, perf_mode=DR,
                                         tile_position=(0, 0),
                                         skip_group_check=True)
                    for p in range(NP):
                        nc.tensor.matmul(b_z1[0:32, :],
                                         lhsT=hT8ap(0, p, t1 % RING),
                                         rhs=w8sl(w8i1, 1, p), start=False,
                                         stop=False, perf_mode=DR,
                                         tile_position=(0, 0),
                                         skip_group_check=True)
                # B half-banks: one-hot bias init (ni1/nh0/nh1) + x-n + wi1n
                for half, bBh in [(0, bB0), (1, bB1)]:
                    hs = slice(half * HH, half * HH + HH)
                    nc.tensor.matmul(bBh[:, 0:HH], lhsT=onehotB3[:, :],
                                     rhs=biasB3[:, hs], start=True, stop=False,
                                     tile_position=(0, 0),
                                     skip_group_check=True)
                    if l0:
                        xt = xw[0:2, BL * t0: BL * (t0 + 1)]
                        nc.tensor.matmul(bBh[0:32, 0:HH], lhsT=xt,
                                         rhs=wx2[0:2, 2 * H + half * HH:
                                                 2 * H + half * HH + HH],
                                         start=False, stop=False,
                                         tile_position=(0, 0),
                                         skip_group_check=True)
                    if l1:
                        hb = hTb[0][t1 % RING]
                        for c in range(NK):
                            nc.tensor.matmul(bBh[32:64, 0:HH],
                                             lhsT=hb[:, BL * c:BL * (c + 1)],
                                             rhs=wnsl(2, c, half),
                                             start=False, stop=False,
                                             tile_position=(0, 32),
                                             skip_group_check=True)

            def emit_late(s):
                """wh0/wh1 matmuls (depend on superstep s-1's ring copies);
                set the stop flags."""
                l0 = s < T
                l1 = s >= skew
                t0, t1 = s, s - skew
                b_r0, b_r1, b_z0, b_z1, bB0, bB1 = banks[s]
                # r/z DR mms, pair-major
                groups = []  # (bank, lhsT-list)
                if l0 and t0 >= 1:
                    groups.append((b_r0, [(hT8ap(0, p, (t0 - 1) % RING),
                                           w8sl(w8h0, 0, p)) for p in range(NP)]))
                    groups.append((b_z0, [(hT8ap(0, p, (t0 - 1) % RING),
                                           w8sl(w8h0, 1, p)) for p in range(NP)]))
                elif l0:
                    groups.append((b_r0, []))
                    groups.append((b_z0, []))
                if l1 and t1 >= 1:
                    groups.append((b_r1, [(hT8ap(1, p, (t1 - 1) % RING),
                                           w8sl(w8h1, 0, p)) for p in range(NP)]))
                    groups.append((b_z1, [(hT8ap(1, p, (t1 - 1) % RING),
                                           w8sl(w8h1, 1, p)) for p in range(NP)]))
                elif l1:
                    groups.append((b_r1, []))
                    groups.append((b_z1, []))
                for bank, mms in groups:
                    if not mms:
                        nc.tensor.matmul(bank[0:32, :], lhsT=ones_lhs,
                                         rhs=zeros_rhs, start=False, stop=True,
                                         tile_position=(0, 0),
                                         skip_group_check=True)
                        continue
                    for i, (lh, rh) in enumerate(mms):
                        nc.tensor.matmul(bank[0:32, :], lhsT=lh, rhs=rh,
                                         start=False, stop=(i == len(mms) - 1),
                                         perf_mode=DR, tile_position=(0, 0),
                                         skip_group_check=True)
                # nh halves (bf16)
                for half, bBh in [(0, bB0), (1, bB1)]:
                    b_mms = []
                    if l0 and t0 >= 1:
                        hb = hTb[0][(t0 - 1) % RING]
                        for c in range(NK):
                            b_mms.append((hb[:, BL * c:BL * (c + 1)],
                                          wnsl(0, c, half), 64))
                    if l1 and t1 >= 1:
                        hb = hTb[1][(t1 - 1) % RING]
                        for c in range(NK):
                            b_mms.append((hb[:, BL * c:BL * (c + 1)],
                                          wnsl(1, c, half), 96))
                    for i, (lh, rh, ps) in enumerate(b_mms):
                        nc.tensor.matmul(bBh[ps:ps + 32, 0:HH],
                                         lhsT=lh, rhs=rh, start=False,
                                         stop=(i == len(b_mms) - 1),
                                         tile_position=(0, ps),
                                         skip_group_check=True)
                    if not b_mms:
                        nc.tensor.matmul(bBh[0:32, 0:HH], lhsT=ones_lhs,
                                         rhs=zeros_rhs[0:1, 0:HH],
                                         start=False, stop=True,
                                         tile_position=(0, 0),
                                         skip_group_check=True)

            emit_early(0)
            for s in range(n_super):
                l0 = s < T
                l1 = s >= skew
                t0, t1 = s, s - skew
                par = s % 2
                b_r0, b_r1, b_z0, b_z1, bB0, bB1 = banks[s]

                emit_late(s)

                rz = scratch.tile([128, H], BF16, tag="rz")
                tmp = scratch.tile([64, H], BF16, tag="tmp")
                u = scratch.tile([64, H], BF16, tag="u")
                nn_ = scratch.tile([64, H], BF16, tag="nn")
                dd = scratch.tile([128, H], BF16, tag="dd")
                ee = scratch.tile([64, H], BF16, tag="ee")
                hnew = h_sb[par]
                hold = h_sb[1 - par]

                # per-gate sigmoids into the shared rz layout
                # (r0@0 r1@32 zbar0@64 zbar1@96)
                if l0:
                    nc.scalar.activation(rz[0:32, :], b_r0[0:32, :],
                                         AF.Sigmoid, scale=ISCALE)
                    nc.scalar.activation(rz[64:96, :], b_z0[0:32, :],
                                         AF.Sigmoid, scale=-ISCALE)
                if l1:
                    nc.scalar.activation(rz[32:64, :], b_r1[0:32, :],
                                         AF.Sigmoid, scale=ISCALE)
                    nc.scalar.activation(rz[96:128, :], b_z1[0:32, :],
                                         AF.Sigmoid, scale=-ISCALE)
                lo, hi = (0 if l0 else 32), (64 if l1 else 32)

                halves = [(0, bB0), (1, bB1)]
                # DVE: tmp/u straight from psum, 1/64 scale folded in
                for half, bBh in halves:
                    hc = slice(half * HH, half * HH + HH)
                    nc.vector.scalar_tensor_tensor(
                        out=tmp[lo:hi, hc], in0=bBh[64 + lo:64 + hi, 0:HH],
                        scalar=ISCALE, in1=rz[lo:hi, hc],
                        op0=ALU.mult, op1=ALU.mult)
                    nc.vector.scalar_tensor_tensor(
                        out=u[lo:hi, hc], in0=bBh[lo:hi, 0:HH],
                        scalar=ISCALE, in1=tmp[lo:hi, hc],
                        op0=ALU.mult, op1=ALU.add)
                # ACT: tanh per half; Pool: d, e; DVE: h'
                for half, _ in halves:
                    hc = slice(half * HH, half * HH + HH)
                    nc.scalar.activation(nn_[lo:hi, hc], u[lo:hi, hc], AF.Tanh)
                    nc.gpsimd.tensor_sub(out=dd[64 + lo:64 + hi, hc],
                                         in0=nn_[lo:hi, hc],
                                         in1=hold[lo:hi, hc])
                    nc.gpsimd.tensor_mul(out=ee[lo:hi, hc],
                                         in0=dd[64 + lo:64 + hi, hc],
                                         in1=rz[64 + lo:64 + hi, hc])
                    nc.vector.tensor_add(out=hnew[lo:hi, hc],
                                         in0=ee[lo:hi, hc],
                                         in1=hold[lo:hi, hc])

                # ---- early matmuls of next superstep (keep PE busy) ----
                emit_early(s + 1)

                # ---- transposes + ring copies, per half ----
                trpsX = psp.tile([128, 1024], BF16, tag="trpsX")
                trpsY = psp.tile([128, 1024], BF16, tag="trpsY")
                for half, _ in halves:
                    for (active, base, l) in [(l0, 0, 0), (l1, 32, 1)]:
                        if not active:
                            continue
                        tstep = t0 if l == 0 else t1
                        blk = (l + half) % 2          # X, Y, Y, X
                        tr = trpsX if blk == 0 else trpsY
                        boff = half * 256
                        for j, c in enumerate((2 * half, 2 * half + 1)):
                            nc.tensor.transpose(
                                tr[:, boff + BL * j: boff + BL * (j + 1)],
                                hnew[base:base + 32, 128 * c:128 * (c + 1)],
                                id4[base:base + 32, :],
                                tile_position=(base, 0),
                            )
                        src = tr[:, boff:boff + 2 * BL]
                        dstb = hTb[l][tstep % RING][:, 2 * BL * half:
                                                    2 * BL * (half + 1)]
                        dst8 = hT8[l][half][tstep % RING][:, :]
                        if l == 0:
                            nc.scalar.activation(dstb, src, AF.Copy)
                            nc.scalar.activation(dst8, src, AF.Copy,
                                                 scale=SCALE)
                        else:
                            nc.vector.tensor_copy(out=dstb, in_=src)
                            nc.vector.tensor_scalar_mul(dst8, src, SCALE)

            # ---- FC ----
            psfc = psp.tile([128, H], F32, tag="bB0")
            hfcT = hTb[1][(T - 1) % RING]
            for c in range(NK):
                nc.tensor.matmul(psfc[0:BL, 0:2],
                                 lhsT=hfcT[:, BL * c:BL * (c + 1)],
                                 rhs=wfc[:, 2 * c:2 * (c + 1)],
                                 start=(c == 0), stop=False,
                                 skip_group_check=True)
            nc.tensor.matmul(psfc[0:BL, 0:2], lhsT=ones_lhs, rhs=fcb[0:1, :],
                             start=False, stop=True, skip_group_check=True)
            out_sb = const.tile([BL, 2], F32)
            nc.vector.tensor_copy(out=out_sb[:], in_=psfc[0:BL, 0:2])
            nc.sync.dma_start(out=out_d, in_=out_sb[:])

    nc.compile()
    return nc


# ---------------- host-side packing ----------------

def pack_inputs(x, Wi0, bi0, Wi_rest, bi_rest, Wh, bh, fc_w, fc_b, n_cores=8):
    B, T = x.shape
    bl = B // n_cores
    assert bl == BL
    S = SCALE
    S2 = S * S

    def w8_pack(W3):  # r/z gates -> [128, 8*H] fp8
        a = np.empty((128, 2, NP, 2, H), np.float32)
        for g in range(2):
            for p in range(NP):
                for ko in range(2):
                    k0 = p * 256 + ko * 128
                    a[:, g, p, ko, :] = W3[g, :, k0:k0 + 128].T * S
        return a.reshape(128, 8 * H).astype(ml_dtypes.float8_e4m3)

    w8h0 = w8_pack(Wh[0])
    w8h1 = w8_pack(Wh[1])
    w8i1 = w8_pack(Wi_rest[0])

    wnb = np.empty((128, 3, NK, H), np.float32)
    for m, W3 in enumerate([Wh[0], Wh[1], Wi_rest[0]]):
        for c in range(NK):
            wnb[:, m, c, :] = W3[2, :, c * 128:(c + 1) * 128].T * S2
    wnb = wnb.reshape(128, 12 * H).astype(ml_dtypes.bfloat16)

    bias_l0 = [bi0[0] + bh[0, 0], bi0[1] + bh[0, 1], bi0[2]]
    bias_l1 = [bi_rest[0, 0] + bh[1, 0], bi_rest[0, 1] + bh[1, 1],
               bi_rest[0, 2]]
    wx2 = np.zeros((2, 3 * H), np.float32)
    for g in range(3):
        wx2[0, g * H:(g + 1) * H] = Wi0[g, :, 0] * S2
        wx2[1, g * H:(g + 1) * H] = bias_l0[g] * S2
    wx2 = wx2.astype(ml_dtypes.bfloat16)

    biasA = np.concatenate([bias_l1[0] * S2, bias_l1[1] * S2]).reshape(
        1, 2 * H).astype(ml_dtypes.bfloat16)
    # rows: nh0 (bh0n), nh1 (bh1n), ni1 (bi1n)
    biasB3 = np.stack([bh[0, 2] * S2, bh[1, 2] * S2,
                       bias_l1[2] * S2]).astype(ml_dtypes.bfloat16)

    wfc = fc_w.T.reshape(NK, 128, 2).transpose(1, 0, 2)
    wfc = np.ascontiguousarray(wfc).reshape(128, 8).astype(ml_dtypes.bfloat16)
    fcb = fc_b.reshape(1, 2).astype(ml_dtypes.bfloat16)

    onehotB3 = np.zeros((3, 128), np.float32)
    onehotB3[0, 64:96] = 1.0   # nh0
    onehotB3[1, 96:128] = 1.0  # nh1
    onehotB3[2, 32:64] = 1.0   # ni1
    onehotB3 = onehotB3.astype(ml_dtypes.bfloat16)

    in_maps = []
    for c in range(n_cores):
        xc = x[c * bl:(c + 1) * bl, :]       # [32, T]
        xw = np.empty((2, T * bl), np.float32)
        xw[0] = xc.T.reshape(-1)
        xw[1] = 1.0
        in_maps.append({
            "xw": xw.astype(ml_dtypes.bfloat16),
            "w8h0": w8h0, "w8h1": w8h1, "w8i1": w8i1, "wnb": wnb,
            "wx2": wx2, "biasA": biasA, "biasB3": biasB3,
            "wfc": wfc, "fcb": fcb, "onehotB3": onehotB3,
        })
    return in_maps


def unpack_outputs(results):
    return np.concatenate([r["out"] for r in results], axis=0)


# ---------------- public entry point ----------------
_CACHED = {}


def _get_nc(T):
    if T not in _CACHED:
        _CACHED[T] = build_gru(T=T)
    return _CACHED[T]


def kernel(x, Wi0, bi0, Wi_rest, bi_rest, Wh, bh, fc_w, fc_b):
    """Full-input 2-layer GRU (B=256, H=512) on 8 NeuronCores.

    Data-parallel over batch (32/core), weights replicated. fp8 DoubleRow
    r/z matmuls (per-gate psum banks), bf16 n-gate, H-split n-path.
    """
    from concourse.bass_utils import run_bass_kernel_spmd
    x = np.asarray(x); Wi0 = np.asarray(Wi0); bi0 = np.asarray(bi0)
    Wi_rest = np.asarray(Wi_rest); bi_rest = np.asarray(bi_rest)
    Wh = np.asarray(Wh); bh = np.asarray(bh)
    fc_w = np.asarray(fc_w); fc_b = np.asarray(fc_b)
    T = x.shape[1]
    nc = _get_nc(T)
    in_maps = pack_inputs(x, Wi0, bi0, Wi_rest, bi_rest, Wh, bh, fc_w, fc_b)
    res = run_bass_kernel_spmd(nc, in_maps, core_ids=list(range(8)))
    return unpack_outputs(res.results).astype(np.float32)
